# revision 25
# baseline (speedup 1.0000x reference)
"""Binarized 3x3 conv (XNOR-style): sign(conv2d(sign(x), sign(w)) + b).

Full-input contract: kernel(x=[32,256,56,56]f32, weight=[256,256,3,3]f32,
bias=[256]f32) -> [32,256,56,56]f32.

Strategy: data-parallel over batch across 8 NeuronCores (4 images/core).
Per core:
  - sign(x) encoded as +/-0.5 (exact: is_ge -> {0,1}, subtract 0.5) on DVE
    into 57-wide shared-pad rows (one zero col between consecutive rows
    serves as right-pad of row r AND left-pad of row r+1), fp8e4.
  - the padded image is stored as 7 independent GROUP tiles per image, one
    per 8-output-row matmul block (10 padded rows each, rows duplicated at
    block seams).  A DoubleRow rhs access pattern spans [ci0 .. ci1+456] of
    whatever buffer holds the pair, so small per-block tiles keep each
    matmul's dependency footprint to exactly its own two sign chunks —
    the first matmul can start as soon as ~20 rows have landed.
  - sign(w) prepped on host as +/-1 in [c_partition, kg, tap, pair, k].
  - conv = 9 tap-shifted matmuls per 8-row block (fp8 DoubleRow,
    contract=256, free=8*57=456) accumulating into PSUM; products are
    +/-0.5 with f32 accumulation, so psum == conv/2 exactly.
  - evacuation on the Scalar (ACT) engine: sign(psum [+ b/2]) -> fp8
    {-1,0,+1}, exact (sign(0)=0 matches the reference for zero conv sums;
    for nonzero bias conv+b is never exactly 0). Written compacted into a
    per-(img,kg) [128, 3136] fp8 plane, stored as two contiguous ~200KB
    DMAs via the scalar engine's SWDGE queues. The host widens fp8 -> f32
    (values +/-1/0 are exact in fp8e4).
  - PE HAM warmup matmuls sized to cover [engine-start, first-data-ready]
    so the 2.4 GHz clock gate is open when real matmuls begin and the PE
    never idles (idling resets the p-state).
"""

import numpy as np

import concourse.bacc as bacc
import concourse.mybir as mybir
import concourse.tile as tile
from concourse.bass_utils import run_bass_kernel_spmd

N_CORES = 8
N_PER = 4          # images per core
C = 256            # input channels
K = 256            # output channels
H = W = 56
WP = 57            # shared-pad row width
RB = 8             # output rows per matmul tile
F = RB * WP        # 456 matmul free size (8 rows x 57, last col of each row garbage)
NBLK = H // RB     # 7 row blocks (= group tiles) per image
GROWS = RB + 2     # padded rows per group tile
GPAD = 576         # 1 + 10*57 = 571 -> pad 576 (ci-pair stride, %16)

WARM_N = 12        # HAM warmup matmuls (tuned against the input-ready time)

_cache = {}


def _grp_rows(g):
    """Data rows fetched for group g (padded rows 8g..8g+9).

    Returns (first data row, row count, local padded-row index of that row):
    padded row p holds data row p-1, group-local row l = padded row 8g+l.
    """
    r0 = max(0, 8 * g - 1)
    r1 = min(H - 1, 8 * g + 8)
    return r0, r1 - r0 + 1, r0 + 1 - 8 * g


def _build(with_bias):
    dt = mybir.dt
    xdt = dt.float8e4
    nc = bacc.Bacc()
    x_d = nc.declare_dram_parameter("xs", [N_PER, C, H, W], dt.float32, isOutput=False)
    wfree = 9 * 2 * 256
    w_d = nc.declare_dram_parameter("wsgn", [128, wfree], xdt, isOutput=False)
    if with_bias:
        b_d = nc.declare_dram_parameter("bhalf", [128, 2], dt.float32, isOutput=False)
    o_d = nc.declare_dram_parameter("out", [N_PER, K, H, W], xdt, isOutput=True)

    with tile.TileContext(nc) as tc:
        with (
            tc.tile_pool(name="wpool", bufs=1) as wpool,
            tc.tile_pool(name="xsgn", bufs=NBLK * N_PER) as xsgn_pool,
            tc.tile_pool(name="xf32", bufs=6) as xf_pool,
            tc.tile_pool(name="osb", bufs=4) as o_pool,
            tc.tile_pool(name="psum", bufs=8, space="PSUM") as p_pool,
        ):
            # Warm the PE HAM clock gate while the first rows stream in.
            # Results discarded; source is a small zeroed tile.  Idling
            # resets the p-state, so the count is tuned to end at data-ready.
            wsrc = wpool.tile([128, 512], xdt)
            nc.vector.memset(wsrc[:], 0.0)
            warm = p_pool.tile([128, F], dt.float32, tag="ps")
            for _ in range(WARM_N):
                nc.tensor.matmul(
                    warm[:], wsrc[:, 0:128], wsrc[:, 0:F],
                    start=True, stop=True,
                )
            w_sb = wpool.tile([128, wfree], xdt)
            if with_bias:
                b_sb = wpool.tile([128, 2], dt.float32)
                nc.sync.dma_start(b_sb[:], b_d[:])

            # per-(image, block) group tiles + border zeroing (interiors get
            # overwritten by the sign writes; only pads need memset)
            grp = {}
            xv = x_d[:].rearrange("n c h w -> n c (h w)")
            for n in range(N_PER):
                for g in range(NBLK):
                    xt = xsgn_pool.tile([128, 2 * GPAD], xdt, tag="xg",
                                        name=f"xg{n}_{g}")
                    grp[(n, g)] = xt
                    for ci in range(2):
                        o = ci * GPAD
                        if g == 0:
                            # leading pad byte + top pad row
                            nc.gpsimd.memset(xt[:, o: o + 1 + WP], 0.0)
                            v = xt[:, o + 1 + WP: o + 1 + GROWS * WP].rearrange(
                                "p (h w) -> p h w", w=WP)[:, :, 56:57]
                            nc.gpsimd.memset(v, 0.0)
                            nc.gpsimd.memset(xt[:, o + 1 + GROWS * WP: o + GPAD], 0.0)
                        elif g == NBLK - 1:
                            nc.gpsimd.memset(xt[:, o: o + 1], 0.0)
                            v = xt[:, o + 1: o + 1 + (GROWS - 1) * WP].rearrange(
                                "p (h w) -> p h w", w=WP)[:, :, 56:57]
                            nc.gpsimd.memset(v, 0.0)
                            # bottom pad row + tail
                            nc.gpsimd.memset(
                                xt[:, o + 1 + (GROWS - 1) * WP: o + GPAD], 0.0)
                        else:
                            nc.gpsimd.memset(xt[:, o: o + 1], 0.0)
                            v = xt[:, o + 1: o + 1 + GROWS * WP].rearrange(
                                "p (h w) -> p h w", w=WP)[:, :, 56:57]
                            nc.gpsimd.memset(v, 0.0)
                            nc.gpsimd.memset(xt[:, o + 1 + GROWS * WP: o + GPAD], 0.0)

            xvp = x_d[:].rearrange("n (i c) h w -> n c i (h w)", i=2)

            def emit_chunk(n, g, eng=None):
                # one DMA brings both ci halves of a group (one completion
                # semaphore per group), then one sign op covers both
                r0, nr, l0 = _grp_rows(g)
                xt = grp[(n, g)]
                xf = xf_pool.tile([128, 2 * nr * W], dt.float32, tag="xf32",
                                  name=f"xf_{n}_{g}")
                (eng or nc.sync).dma_start(
                    xf[:].rearrange("p (i f) -> p i f", i=2),
                    xvp[n, :, :, r0 * W: (r0 + nr) * W],
                )
                dst = (
                    xt[:].rearrange("p (i f) -> p i f", i=2)
                    [:, :, 1: 1 + GROWS * WP]
                    .rearrange("p i (h w) -> p i h w", w=WP)
                    [:, :, l0: l0 + nr, 0:56]
                )
                src = xf[:].rearrange("p (i h w) -> p i h w", i=2, h=nr)
                # (x>=0 -> {0,1}) - 0.5 = +/-0.5, exact
                nc.vector.tensor_scalar(
                    dst, src, 0.0, 0.5, mybir.AluOpType.is_ge,
                    mybir.AluOpType.subtract,
                )

            # group 0 of image 0 first (both ci), then w-kg0 in two pieces —
            # the first matmul gates only on taps 0-2 (768B) while taps 3-8
            # land during the first accumulation — then the remaining
            # image-0 groups, w-kg1, then images 1-3
            # group 0's load goes out on the gpsimd engine's SWDGE: that
            # sequencer starts ~2.5us before the sync sequencer, so the
            # first group's data (the thing gating the first real matmul)
            # leaves earliest; everything else uses sync/HWDGE
            emit_chunk(0, 0, eng=nc.gpsimd)
            nc.sync.dma_start(w_sb[:, 0: 3 * 256], w_d[:, 0: 3 * 256])
            nc.sync.dma_start(w_sb[:, 3 * 256: wfree // 2],
                              w_d[:, 3 * 256: wfree // 2])
            nc.sync.dma_start(w_sb[:, wfree // 2:], w_d[:, wfree // 2:])
            for g in range(1, NBLK):
                emit_chunk(0, g)
            for n in range(1, N_PER):
                for g in range(NBLK):
                    emit_chunk(n, g)

            wv = w_sb[:].rearrange("p (g t i k) -> p g t i k", g=2, t=9, i=2)

            planes = {}

            def emit_rb(n, kg, rb):
                ps = p_pool.tile([128, F], dt.float32, tag="ps",
                                 name=f"ps{kg}_{n}_{rb}")
                xt = grp[(n, rb)]
                xp = xt[:].rearrange("p (i f) -> p i f", i=2)
                for tap in range(9):
                    ty, tx = tap // 3, tap % 3
                    base = ty * WP + tx
                    nc.tensor.matmul(
                        ps[:], wv[:, kg, tap, :, :], xp[:, :, base: base + F],
                        start=(tap == 0), stop=(tap == 8),
                        perf_mode=mybir.MatmulPerfMode.DoubleRow,
                    )
                emit_evac(n, kg, rb, ps)

            def emit_evac(n, kg, rb, ps):
                # sign(psum [+ b/2]) -> fp8 on the ACT engine, compacting the
                # valid 8x56 of the 8x57 psum span into the output plane
                if (n, kg) not in planes:
                    planes[(n, kg)] = o_pool.tile(
                        [128, H * W], xdt, tag="osb", name=f"osb{kg}_{n}")
                plane = planes[(n, kg)]
                psv = ps[:].rearrange("p (r c) -> p r c", r=RB)[:, :, 0:W]
                ov = plane[:, rb * RB * W: (rb + 1) * RB * W].rearrange(
                    "p (r c) -> p r c", r=RB)
                bias = b_sb[:, kg: kg + 1] if with_bias else 0.0
                nc.scalar.sign(ov, psv, bias)
                # three contiguous stores per (img, kg) plane (shrinks the
                # drain after the very last matmul); SWDGE via the scalar
                # engine keeps HWDGE loads unblocked
                splits = {3: (0, 4), 5: (4, 6), NBLK - 1: (6, NBLK)}
                if rb in splits:
                    b0, b1 = splits[rb]
                    dst = o_d[n, kg * 128:(kg + 1) * 128,
                              b0 * RB: b1 * RB].rearrange("k h w -> k (h w)")
                    nc.scalar.dma_start(
                        dst, plane[:, b0 * RB * W: b1 * RB * W])

            # rb-major, kg-minor: each group tile feeds two consecutive
            # matmul blocks, so the input DMA+sign pipeline only has to
            # deliver a group every ~3.5us instead of every ~1.7us
            for n in range(N_PER):
                for rb in range(NBLK):
                    for kg in range(2):
                        emit_rb(n, kg, rb)

    nc.finalize()
    return nc


def _prep_weights(weight):
    sgn = np.sign(weight.astype(np.float32))
    w6 = sgn.reshape(2, 128, 2, 128, 3, 3)     # [kg, kk, i, p, ty, tx]
    arr = w6.transpose(3, 0, 4, 5, 2, 1)       # [p, kg, ty, tx, i, kk]
    arr = np.ascontiguousarray(arr).reshape(128, 9 * 2 * 256)
    return arr.astype(mybir.dt.np(mybir.dt.float8e4))


def kernel(x, weight, bias, _profile=False, _trace_kwargs=None):
    x = np.asarray(x, dtype=np.float32)
    weight = np.asarray(weight, dtype=np.float32)
    bias = np.asarray(bias, dtype=np.float32)
    assert x.shape == (N_CORES * N_PER, C, H, W), x.shape
    assert weight.shape == (K, C, 3, 3), weight.shape
    assert bias.shape == (K,), bias.shape
    with_bias = bool(np.any(bias != 0.0))

    if with_bias not in _cache:
        _cache[with_bias] = _build(with_bias)
    nc = _cache[with_bias]

    wsgn = _prep_weights(weight)
    in_maps = []
    for c in range(N_CORES):
        m = {
            "xs": np.ascontiguousarray(x[c * N_PER:(c + 1) * N_PER]),
            "wsgn": wsgn,
        }
        if with_bias:
            m["bhalf"] = np.ascontiguousarray(
                (bias.reshape(2, 128).T * 0.5).astype(np.float32)
            )
        in_maps.append(m)

    res = run_bass_kernel_spmd(
        nc, in_maps, core_ids=list(range(N_CORES)),
        trace=_profile, **(_trace_kwargs or {}),
    )
    out = np.concatenate(
        [res.results[c]["out"] for c in range(N_CORES)], axis=0
    ).astype(np.float32)
    if _profile:
        kernel.last_exec_ns = res.exec_time_ns
        kernel.last_results = res
    return out


# revision 26
# speedup vs baseline: 1.1269x; 1.1269x over previous
"""Binarized 3x3 conv (XNOR-style): sign(conv2d(sign(x), sign(w)) + b).

Full-input contract: kernel(x=[32,256,56,56]f32, weight=[256,256,3,3]f32,
bias=[256]f32) -> [32,256,56,56]f32.

Strategy: data-parallel over batch across 8 NeuronCores (4 images/core).
Per core:
  - sign(x) encoded as +/-0.5 (exact: is_ge -> {0,1}, subtract 0.5) on DVE
    into 57-wide shared-pad rows (one zero col between consecutive rows
    serves as right-pad of row r AND left-pad of row r+1), fp8e4.
  - the padded image is stored as 7 independent GROUP tiles per image, one
    per 8-output-row matmul block (10 padded rows each, rows duplicated at
    block seams).  A DoubleRow rhs access pattern spans [ci0 .. ci1+456] of
    whatever buffer holds the pair, so small per-block tiles keep each
    matmul's dependency footprint to exactly its own two sign chunks —
    the first matmul can start as soon as ~20 rows have landed.
  - sign(w) prepped on host as +/-1 in [c_partition, kg, tap, pair, k].
  - conv = 9 tap-shifted matmuls per 8-row block (fp8 DoubleRow,
    contract=256, free=8*57=456) accumulating into PSUM; products are
    +/-0.5 with f32 accumulation, so psum == conv/2 exactly.
  - evacuation on the Scalar (ACT) engine: sign(psum [+ b/2]) -> fp8
    {-1,0,+1}, exact (sign(0)=0 matches the reference for zero conv sums;
    for nonzero bias conv+b is never exactly 0). Written compacted into a
    per-(img,kg) [128, 3136] fp8 plane, stored as two contiguous ~200KB
    DMAs via the scalar engine's SWDGE queues. The host widens fp8 -> f32
    (values +/-1/0 are exact in fp8e4).
  - PE HAM warmup matmuls sized to cover [engine-start, first-data-ready]
    so the 2.4 GHz clock gate is open when real matmuls begin and the PE
    never idles (idling resets the p-state).
"""

import numpy as np

import concourse.bacc as bacc
import concourse.mybir as mybir
import concourse.tile as tile
from concourse.bass_utils import run_bass_kernel_spmd

N_CORES = 8
N_PER = 4          # images per core
C = 256            # input channels
K = 256            # output channels
H = W = 56
WP = 57            # shared-pad row width
RB = 8             # output rows per matmul tile
F = RB * WP        # 456 matmul free size (8 rows x 57, last col of each row garbage)
NBLK = H // RB     # 7 row blocks (= group tiles) per image
GROWS = RB + 2     # padded rows per group tile
GPAD = 576         # 1 + 10*57 = 571 -> pad 576 (ci-pair stride, %16)

WARM_N = 12        # HAM warmup matmuls (tuned against the input-ready time)

_cache = {}


def _grp_rows(g):
    """Data rows fetched for group g (padded rows 8g..8g+9).

    Returns (first data row, row count, local padded-row index of that row):
    padded row p holds data row p-1, group-local row l = padded row 8g+l.
    """
    r0 = max(0, 8 * g - 1)
    r1 = min(H - 1, 8 * g + 8)
    return r0, r1 - r0 + 1, r0 + 1 - 8 * g


def _build(with_bias):
    dt = mybir.dt
    xdt = dt.float8e4
    nc = bacc.Bacc()
    x_d = nc.declare_dram_parameter("xs", [N_PER, C, H, W], dt.float32, isOutput=False)
    wfree = 9 * 2 * 256
    w_d = nc.declare_dram_parameter("wsgn", [128, wfree], xdt, isOutput=False)
    if with_bias:
        b_d = nc.declare_dram_parameter("bhalf", [128, 2], dt.float32, isOutput=False)
    o_d = nc.declare_dram_parameter("out", [N_PER, K, H, W], xdt, isOutput=True)

    with tile.TileContext(nc) as tc:
        with (
            tc.tile_pool(name="wpool", bufs=1) as wpool,
            tc.tile_pool(name="xsgn", bufs=NBLK * N_PER) as xsgn_pool,
            tc.tile_pool(name="xf32", bufs=6) as xf_pool,
            tc.tile_pool(name="osb", bufs=4) as o_pool,
            tc.tile_pool(name="psum", bufs=8, space="PSUM") as p_pool,
        ):
            # Warm the PE HAM clock gate while the first rows stream in.
            # Results discarded; source is a small zeroed tile.  Idling
            # resets the p-state, so the count is tuned to end at data-ready.
            wsrc = wpool.tile([128, 512], xdt)
            nc.vector.memset(wsrc[:], 0.0)
            warm = p_pool.tile([128, F], dt.float32, tag="ps")
            for _ in range(WARM_N):
                nc.tensor.matmul(
                    warm[:], wsrc[:, 0:128], wsrc[:, 0:F],
                    start=True, stop=True,
                )
            w_sb = wpool.tile([128, wfree], xdt)
            if with_bias:
                b_sb = wpool.tile([128, 2], dt.float32)
                nc.sync.dma_start(b_sb[:], b_d[:])

            # per-(image, block) group tiles + border zeroing (interiors get
            # overwritten by the sign writes; only pads need memset)
            grp = {}
            xv = x_d[:].rearrange("n c h w -> n c (h w)")
            for n in range(N_PER):
                for g in range(NBLK):
                    xt = xsgn_pool.tile([128, 2 * GPAD], xdt, tag="xg",
                                        name=f"xg{n}_{g}")
                    grp[(n, g)] = xt
                    for ci in range(2):
                        o = ci * GPAD
                        if g == 0:
                            # leading pad byte + top pad row
                            nc.gpsimd.memset(xt[:, o: o + 1 + WP], 0.0)
                            v = xt[:, o + 1 + WP: o + 1 + GROWS * WP].rearrange(
                                "p (h w) -> p h w", w=WP)[:, :, 56:57]
                            nc.gpsimd.memset(v, 0.0)
                            nc.gpsimd.memset(xt[:, o + 1 + GROWS * WP: o + GPAD], 0.0)
                        elif g == NBLK - 1:
                            nc.gpsimd.memset(xt[:, o: o + 1], 0.0)
                            v = xt[:, o + 1: o + 1 + (GROWS - 1) * WP].rearrange(
                                "p (h w) -> p h w", w=WP)[:, :, 56:57]
                            nc.gpsimd.memset(v, 0.0)
                            # bottom pad row + tail
                            nc.gpsimd.memset(
                                xt[:, o + 1 + (GROWS - 1) * WP: o + GPAD], 0.0)
                        else:
                            nc.gpsimd.memset(xt[:, o: o + 1], 0.0)
                            v = xt[:, o + 1: o + 1 + GROWS * WP].rearrange(
                                "p (h w) -> p h w", w=WP)[:, :, 56:57]
                            nc.gpsimd.memset(v, 0.0)
                            nc.gpsimd.memset(xt[:, o + 1 + GROWS * WP: o + GPAD], 0.0)

            xvp = x_d[:].rearrange("n (i c) h w -> n c i (h w)", i=2)

            def emit_chunk(n, g, eng=None):
                # one DMA brings both ci halves of a group (one completion
                # semaphore per group), then one sign op covers both
                r0, nr, l0 = _grp_rows(g)
                xt = grp[(n, g)]
                xf = xf_pool.tile([128, 2 * nr * W], dt.float32, tag="xf32",
                                  name=f"xf_{n}_{g}")
                (eng or nc.sync).dma_start(
                    xf[:].rearrange("p (i f) -> p i f", i=2),
                    xvp[n, :, :, r0 * W: (r0 + nr) * W],
                )
                dst = (
                    xt[:].rearrange("p (i f) -> p i f", i=2)
                    [:, :, 1: 1 + GROWS * WP]
                    .rearrange("p i (h w) -> p i h w", w=WP)
                    [:, :, l0: l0 + nr, 0:56]
                )
                src = xf[:].rearrange("p (i h w) -> p i h w", i=2, h=nr)
                # (x>=0 -> {0,1}) - 0.5 = +/-0.5, exact
                nc.vector.tensor_scalar(
                    dst, src, 0.0, 0.5, mybir.AluOpType.is_ge,
                    mybir.AluOpType.subtract,
                )

            # group 0 of image 0 first (both ci), then w-kg0 in two pieces —
            # the first matmul gates only on taps 0-2 (768B) while taps 3-8
            # land during the first accumulation — then the remaining
            # image-0 groups, w-kg1, then images 1-3
            emit_chunk(0, 0)
            nc.sync.dma_start(w_sb[:, 0: 3 * 256], w_d[:, 0: 3 * 256])
            nc.sync.dma_start(w_sb[:, 3 * 256: wfree // 2],
                              w_d[:, 3 * 256: wfree // 2])
            nc.sync.dma_start(w_sb[:, wfree // 2:], w_d[:, wfree // 2:])
            for g in range(1, NBLK):
                emit_chunk(0, g)
            for n in range(1, N_PER):
                for g in range(NBLK):
                    emit_chunk(n, g)

            wv = w_sb[:].rearrange("p (g t i k) -> p g t i k", g=2, t=9, i=2)

            planes = {}

            def emit_rb(n, kg, rb):
                ps = p_pool.tile([128, F], dt.float32, tag="ps",
                                 name=f"ps{kg}_{n}_{rb}")
                xt = grp[(n, rb)]
                xp = xt[:].rearrange("p (i f) -> p i f", i=2)
                for tap in range(9):
                    ty, tx = tap // 3, tap % 3
                    base = ty * WP + tx
                    nc.tensor.matmul(
                        ps[:], wv[:, kg, tap, :, :], xp[:, :, base: base + F],
                        start=(tap == 0), stop=(tap == 8),
                        perf_mode=mybir.MatmulPerfMode.DoubleRow,
                    )
                emit_evac(n, kg, rb, ps)

            def emit_evac(n, kg, rb, ps):
                # sign(psum [+ b/2]) -> fp8 on the ACT engine, compacting the
                # valid 8x56 of the 8x57 psum span into the output plane
                if (n, kg) not in planes:
                    planes[(n, kg)] = o_pool.tile(
                        [128, H * W], xdt, tag="osb", name=f"osb{kg}_{n}")
                plane = planes[(n, kg)]
                psv = ps[:].rearrange("p (r c) -> p r c", r=RB)[:, :, 0:W]
                ov = plane[:, rb * RB * W: (rb + 1) * RB * W].rearrange(
                    "p (r c) -> p r c", r=RB)
                bias = b_sb[:, kg: kg + 1] if with_bias else 0.0
                nc.scalar.sign(ov, psv, bias)
                # three contiguous stores per (img, kg) plane (shrinks the
                # drain after the very last matmul); SWDGE via the scalar
                # engine keeps HWDGE loads unblocked
                splits = {3: (0, 4), 5: (4, 6), NBLK - 1: (6, NBLK)}
                if rb in splits:
                    b0, b1 = splits[rb]
                    dst = o_d[n, kg * 128:(kg + 1) * 128,
                              b0 * RB: b1 * RB].rearrange("k h w -> k (h w)")
                    nc.scalar.dma_start(
                        dst, plane[:, b0 * RB * W: b1 * RB * W])

            # rb-major, kg-minor: each group tile feeds two consecutive
            # matmul blocks, so the input DMA+sign pipeline only has to
            # deliver a group every ~3.5us instead of every ~1.7us
            for n in range(N_PER):
                for rb in range(NBLK):
                    for kg in range(2):
                        emit_rb(n, kg, rb)

    nc.finalize()
    return nc


def _prep_weights(weight):
    sgn = np.sign(weight.astype(np.float32))
    w6 = sgn.reshape(2, 128, 2, 128, 3, 3)     # [kg, kk, i, p, ty, tx]
    arr = w6.transpose(3, 0, 4, 5, 2, 1)       # [p, kg, ty, tx, i, kk]
    arr = np.ascontiguousarray(arr).reshape(128, 9 * 2 * 256)
    return arr.astype(mybir.dt.np(mybir.dt.float8e4))


def kernel(x, weight, bias, _profile=False, _trace_kwargs=None):
    x = np.asarray(x, dtype=np.float32)
    weight = np.asarray(weight, dtype=np.float32)
    bias = np.asarray(bias, dtype=np.float32)
    assert x.shape == (N_CORES * N_PER, C, H, W), x.shape
    assert weight.shape == (K, C, 3, 3), weight.shape
    assert bias.shape == (K,), bias.shape
    with_bias = bool(np.any(bias != 0.0))

    if with_bias not in _cache:
        _cache[with_bias] = _build(with_bias)
    nc = _cache[with_bias]

    wsgn = _prep_weights(weight)
    in_maps = []
    for c in range(N_CORES):
        m = {
            "xs": np.ascontiguousarray(x[c * N_PER:(c + 1) * N_PER]),
            "wsgn": wsgn,
        }
        if with_bias:
            m["bhalf"] = np.ascontiguousarray(
                (bias.reshape(2, 128).T * 0.5).astype(np.float32)
            )
        in_maps.append(m)

    res = run_bass_kernel_spmd(
        nc, in_maps, core_ids=list(range(N_CORES)),
        trace=_profile, **(_trace_kwargs or {}),
    )
    out = np.concatenate(
        [res.results[c]["out"] for c in range(N_CORES)], axis=0
    ).astype(np.float32)
    if _profile:
        kernel.last_exec_ns = res.exec_time_ns
        kernel.last_results = res
    return out
